# revision 4
# baseline (speedup 1.0000x reference)
"""Trainium2 Bass kernel for nn_Application_85469849191065 (moe_routing).

Data-parallel over the batch dim N=4096 across 8 NeuronCores (512 samples
per core); the small expert weight banks are replicated to every core.

On-device layout is feature-major (activations stored transposed,
[features, samples]) so every matmul's contraction dim lands on the
partition axis with no per-layer transposes.  The per-sample expert blend
  out_n = sum_e coeff[n,e] * (x_n @ W[e] + b[e])
is computed as scale-input MoE:
  out = sum_e ((coeff_e broadcast) * X^T) contracted with W[e]
so the blend over experts accumulates for free in PSUM, and the bias term
rides a K=8 contraction chunk whose moving operand is coeff^T itself.
Compute dtype is bf16 (weights cast f32->bf16 in-flight by the DMA
engines), accumulation fp32 in PSUM.
"""

import numpy as np

import concourse.bass as bass
import concourse.bacc as bacc
import concourse.mybir as mybir
import concourse.tile as tile
from concourse.bass_utils import run_bass_kernel_spmd

F32 = mybir.dt.float32
BF16 = mybir.dt.bfloat16
AF = mybir.ActivationFunctionType
ALU = mybir.AluOpType

N = 4096
NCORES = 8
NS = N // NCORES  # 512 samples per core
LAT, COND, PH2, E = 64, 219, 40, 8
H, OUT = 512, 171
GIN = PH2 + LAT  # 104
X0R = COND + LAT  # 283

# contraction chunking (per expert)
L0C = 3   # chunks of (128, 91->pad128, 64->pad128) rows
L1C = 5   # 4 full h chunks + shared 64-row latent chunk (reused from L0)
L2C = 4   # 4 full h chunks
NJ0 = E * L0C  # 24 w0 chunks
NJ1 = E * L1C  # 40 w1 chunks
NJ2 = E * L2C  # 32 w2 chunks

_CACHE = {}


def _emit_elu(nc, pool, ps_in, out_tile, bias=0.0):
    """out = elu(ps_in + bias), bf16 out.  elu(x) = min(exp(x)-1, relu(x))."""
    e = pool.tile([128, NS], BF16, tag="elu_e")
    r = pool.tile([128, NS], BF16, tag="elu_r")
    nc.scalar.activation(e[:], ps_in, AF.Exp, bias=bias)
    nc.scalar.activation(r[:], ps_in, AF.Relu, bias=bias)
    nc.vector.scalar_tensor_tensor(out_tile, e[:], -1.0, r[:], ALU.add, ALU.min)


def build():
    nc = bacc.Bacc("TRN2", target_bir_lowering=False, debug=False,
                   num_devices=NCORES)

    # ---- dram parameters (per-core shard shapes) ----
    xg_ext = nc.dram_tensor("xg", [GIN, NS], F32, kind="ExternalInput")
    x0_ext = nc.dram_tensor("x0", [128, L0C, NS], F32, kind="ExternalInput")
    gw1_ext = nc.dram_tensor("gw1", [GIN, 128], F32, kind="ExternalInput")
    gw2_ext = nc.dram_tensor("gw2", [128, 128], F32, kind="ExternalInput")
    gw3_ext = nc.dram_tensor("gw3", [128, E], F32, kind="ExternalInput")
    gb12_ext = nc.dram_tensor("gb12", [128, 2], F32, kind="ExternalInput")
    gb3_ext = nc.dram_tensor("gb3", [E, 1], F32, kind="ExternalInput")
    ident_ext = nc.dram_tensor("ident", [128, 128], F32, kind="ExternalInput")
    w0_ext = nc.dram_tensor("w0", [128, NJ0, H], F32, kind="ExternalInput")
    w1_ext = nc.dram_tensor("w1", [128, NJ1, H], F32, kind="ExternalInput")
    w2_ext = nc.dram_tensor("w2", [128, NJ2, OUT], F32, kind="ExternalInput")
    bh_ext = nc.dram_tensor("bh", [E, 2 * H + OUT], F32, kind="ExternalInput")

    predt_ext = nc.dram_tensor("predt", [OUT, NS], F32, kind="ExternalOutput")
    coeff4_ext = nc.dram_tensor("coeff4", [128, NS // 128, E], F32,
                                kind="ExternalOutput")

    with tile.TileContext(nc) as tc:
        with (
            tc.tile_pool(name="w", bufs=1) as wp,
            tc.tile_pool(name="xs", bufs=56) as xsp,
            tc.tile_pool(name="h", bufs=4) as hp,
            tc.tile_pool(name="elu", bufs=4) as ep,
            tc.tile_pool(name="cb", bufs=1) as cbp,
            tc.tile_pool(name="sm", bufs=1) as smp,
            tc.tile_pool(name="ps", bufs=3, space="PSUM") as psp,
            tc.tile_pool(name="psg", bufs=1, space="PSUM") as psg,
            tc.tile_pool(name="pss", bufs=1, space="PSUM") as pss,
        ):
            # ---- persistent sbuf tiles ----
            GW1 = wp.tile([GIN, 128], BF16)
            GW2 = wp.tile([128, 128], BF16)
            GW3 = wp.tile([128, E], BF16)
            GB12 = wp.tile([128, 2], F32)
            GB3 = wp.tile([E, 1], F32)
            IDT = wp.tile([128, 128], F32)
            XG = wp.tile([GIN, NS], BF16)
            X0 = wp.tile([128, L0C * NS], BF16)
            BH = wp.tile([E, 2 * H + OUT], BF16)
            W0 = wp.tile([128, NJ0 * H], BF16)
            W1 = wp.tile([128, NJ1 * H], BF16)
            W2 = wp.tile([128, NJ2 * OUT], BF16)

            # ---- input DMAs (gpsimd SWDGE casts f32 -> bf16) ----
            nc.gpsimd.dma_start(GW1[:], gw1_ext[:])
            nc.gpsimd.dma_start(GW2[:], gw2_ext[:])
            nc.gpsimd.dma_start(GW3[:], gw3_ext[:])
            nc.sync.dma_start(GB12[:], gb12_ext[:])
            nc.sync.dma_start(GB3[:], gb3_ext[:])
            nc.sync.dma_start(IDT[:], ident_ext[:])
            nc.gpsimd.dma_start(XG[:], xg_ext[:])
            nc.gpsimd.dma_start(X0[:], x0_ext[:])
            nc.gpsimd.dma_start(BH[:], bh_ext[:])
            # weights: split into a few DMAs so matmuls can start early
            for j0, j1 in ((0, 8), (8, 16), (16, 24)):
                nc.gpsimd.dma_start(W0[:, j0 * H:j1 * H], w0_ext[:, j0:j1, :])
            for j0, j1 in ((0, 8), (8, 16), (16, 24), (24, 32), (32, 40)):
                nc.gpsimd.dma_start(W1[:, j0 * H:j1 * H], w1_ext[:, j0:j1, :])
            for j0, j1 in ((0, 16), (16, 32)):
                nc.gpsimd.dma_start(W2[:, j0 * OUT:j1 * OUT], w2_ext[:, j0:j1, :])

            # ---- gate network (feature-major) ----
            pg1 = psg.tile([128, NS], F32, tag="pg")
            nc.tensor.matmul(pg1[:], GW1[:], XG[:], start=True, stop=True)
            G1 = hp.tile([128, NS], BF16, tag="g")
            _emit_elu(nc, ep, pg1[:], G1[:], bias=GB12[:, 0:1])

            pg2 = psg.tile([128, NS], F32, tag="pg")
            nc.tensor.matmul(pg2[:], GW2[:], G1[:], start=True, stop=True)
            G2 = hp.tile([128, NS], BF16, tag="g")
            _emit_elu(nc, ep, pg2[:], G2[:], bias=GB12[:, 1:2])

            pgl = pss.tile([E, NS], F32, tag="pl")
            nc.tensor.matmul(pgl[:], GW3[:], G2[:], start=True, stop=True)
            # exp(logits + b3): softmax numerator, feature-major [E, NS]
            EXPF = smp.tile([E, NS], F32)
            nc.scalar.activation(EXPF[:], pgl[:], AF.Exp, bias=GB3[:])

            # transpose to sample-major [128, 4, E] to normalize per sample
            E4 = smp.tile([128, NS // 128, E], F32)
            for c in range(NS // 128):
                pt = pss.tile([128, E], F32, tag="pt")
                nc.tensor.transpose(pt[:], EXPF[:, c * 128:(c + 1) * 128],
                                    IDT[0:E, 0:E])
                nc.vector.tensor_copy(E4[:, c, :], pt[:])
            S4 = smp.tile([128, NS // 128], F32)
            nc.vector.tensor_reduce(S4[:], E4[:], mybir.AxisListType.X, ALU.add)
            R4 = smp.tile([128, NS // 128], F32)
            nc.vector.reciprocal(R4[:], S4[:])
            C4 = smp.tile([128, NS // 128, E], F32)
            for c in range(NS // 128):
                nc.vector.tensor_scalar_mul(C4[:, c, :], E4[:, c, :],
                                            R4[:, c:c + 1])
            nc.sync.dma_start(coeff4_ext[:], C4[:])

            # transpose coeff back to feature-major [E, NS] in bf16
            CT = smp.tile([E, NS], BF16)
            for c in range(NS // 128):
                ptc = pss.tile([E, 128], F32, tag="ptc")
                nc.tensor.transpose(ptc[:], C4[:, c, :], IDT[:, 0:128])
                nc.vector.tensor_copy(CT[:, c * 128:(c + 1) * 128], ptc[:])

            # flatten coeff^T rows into partition 0, then broadcast each
            # expert's row to all 128 partitions
            CTS = smp.tile([1, E * NS], BF16)
            nc.sync.dma_start(CTS[:], CT[:])
            CB = []
            for e in range(E):
                cb = cbp.tile([128, NS], BF16, tag=f"cb{e}")
                nc.gpsimd.partition_broadcast(cb[:], CTS[0:1, e * NS:(e + 1) * NS])
                CB.append(cb)

            # ---- layer 0 ----
            xs0 = {}
            for e in range(E):
                for c in range(L0C):
                    t = xsp.tile([128, NS], BF16, tag="xs")
                    nc.vector.tensor_mul(t[:], X0[:, c * NS:(c + 1) * NS],
                                         CB[e][:])
                    xs0[(e, c)] = t
            H1 = []
            for og in range(4):
                ps = psp.tile([128, NS], F32, tag="ps")
                nj = 0
                for e in range(E):
                    for c in range(L0C):
                        j = e * L0C + c
                        nc.tensor.matmul(
                            ps[:],
                            W0[:, j * H + og * 128: j * H + og * 128 + 128],
                            xs0[(e, c)][:], start=(nj == 0), stop=False)
                        nj += 1
                nc.tensor.matmul(ps[:], BH[:, og * 128:(og + 1) * 128], CT[:],
                                 start=False, stop=True)
                ht = hp.tile([128, NS], BF16, tag="h1")
                _emit_elu(nc, ep, ps[:], ht[:])
                H1.append(ht)

            # ---- layer 1 ----
            xs1 = {}
            for e in range(E):
                for c in range(4):
                    t = xsp.tile([128, NS], BF16, tag="xs")
                    nc.vector.tensor_mul(t[:], H1[c][:], CB[e][:])
                    xs1[(e, c)] = t
                xs1[(e, 4)] = xs0[(e, 2)]  # scaled latent, reused
            H2 = []
            for og in range(4):
                ps = psp.tile([128, NS], F32, tag="ps")
                nj = 0
                for e in range(E):
                    for c in range(L1C):
                        j = e * L1C + c
                        nc.tensor.matmul(
                            ps[:],
                            W1[:, j * H + og * 128: j * H + og * 128 + 128],
                            xs1[(e, c)][:], start=(nj == 0), stop=False)
                        nj += 1
                nc.tensor.matmul(ps[:], BH[:, H + og * 128: H + (og + 1) * 128],
                                 CT[:], start=False, stop=True)
                ht = hp.tile([128, NS], BF16, tag="h2")
                _emit_elu(nc, ep, ps[:], ht[:])
                H2.append(ht)

            # ---- layer 2 (no activation) ----
            xs2 = {}
            for e in range(E):
                for c in range(L2C):
                    t = xsp.tile([128, NS], BF16, tag="xs")
                    nc.vector.tensor_mul(t[:], H2[c][:], CB[e][:])
                    xs2[(e, c)] = t
            for og, (m0, msz) in enumerate(((0, 128), (128, OUT - 128))):
                ps = psp.tile([128, NS], F32, tag="ps")
                nj = 0
                for e in range(E):
                    for c in range(L2C):
                        j = e * L2C + c
                        nc.tensor.matmul(
                            ps[0:msz, :],
                            W2[:, j * OUT + m0: j * OUT + m0 + msz],
                            xs2[(e, c)][:], start=(nj == 0), stop=False)
                        nj += 1
                nc.tensor.matmul(ps[0:msz, :], BH[:, 2 * H + m0: 2 * H + m0 + msz],
                                 CT[:], start=False, stop=True)
                po = hp.tile([128, NS], F32, tag="po")
                nc.scalar.activation(po[0:msz, :], ps[0:msz, :], AF.Copy)
                nc.sync.dma_start(predt_ext[m0:m0 + msz, :], po[0:msz, :])

    nc.compile()
    return nc


def _prep(inputs):
    """Host-side shard/layout prep -> list of per-core input dicts."""
    lat = np.ascontiguousarray(np.asarray(inputs["latent"], dtype=np.float32))
    cond = np.ascontiguousarray(np.asarray(inputs["condition"], dtype=np.float32))
    ph = np.asarray(inputs["phase"], dtype=np.float32).reshape(N, PH2)
    gw1 = np.asarray(inputs["gate_w1"], dtype=np.float32)
    gw2 = np.asarray(inputs["gate_w2"], dtype=np.float32)
    gw3 = np.asarray(inputs["gate_w3"], dtype=np.float32)
    gb1 = np.asarray(inputs["gate_b1"], dtype=np.float32)
    gb2 = np.asarray(inputs["gate_b2"], dtype=np.float32)
    gb3 = np.asarray(inputs["gate_b3"], dtype=np.float32)
    w0 = np.asarray(inputs["w0"], dtype=np.float32)
    b0 = np.asarray(inputs["b0"], dtype=np.float32)
    w1 = np.asarray(inputs["w1"], dtype=np.float32)
    b1 = np.asarray(inputs["b1"], dtype=np.float32)
    w2 = np.asarray(inputs["w2"], dtype=np.float32)
    b2 = np.asarray(inputs["b2"], dtype=np.float32)

    xg_t = np.ascontiguousarray(np.concatenate([ph, lat], axis=1).T)  # [104, N]
    # x0 chunks: rows (cond 0:128), (cond 128:219 + pad), (lat + pad)
    x0h = np.zeros((128, L0C, N), dtype=np.float32)
    x0h[:, 0, :] = cond.T[0:128]
    x0h[0:COND - 128, 1, :] = cond.T[128:COND]
    x0h[0:LAT, 2, :] = lat.T

    w0h = np.zeros((128, NJ0, H), dtype=np.float32)
    for e in range(E):
        w0h[:, e * L0C + 0] = w0[e, 0:128]
        w0h[0:COND - 128, e * L0C + 1] = w0[e, 128:COND]
        w0h[0:LAT, e * L0C + 2] = w0[e, COND:X0R]
    w1h = np.zeros((128, NJ1, H), dtype=np.float32)
    for e in range(E):
        for c in range(4):
            w1h[:, e * L1C + c] = w1[e, c * 128:(c + 1) * 128]
        w1h[0:LAT, e * L1C + 4] = w1[e, H:H + LAT]
    w2h = np.zeros((128, NJ2, OUT), dtype=np.float32)
    for e in range(E):
        for c in range(L2C):
            w2h[:, e * L2C + c] = w2[e, c * 128:(c + 1) * 128]
    bh = np.concatenate([b0, b1, b2], axis=1)  # [8, 1195]
    gb12 = np.stack([np.broadcast_to(gb1, (128,)),
                     np.broadcast_to(gb2, (128,))], axis=1).copy()  # [128,2]
    ident = np.eye(128, dtype=np.float32)

    shared = {
        "gw1": gw1, "gw2": gw2, "gw3": gw3,
        "gb12": gb12, "gb3": gb3.reshape(E, 1).copy(), "ident": ident,
        "w0h": w0h, "w1h": w1h, "w2h": w2h, "bh": bh,
    }
    in_maps = []
    for ci in range(NCORES):
        s = slice(ci * NS, (ci + 1) * NS)
        in_maps.append({
            "xg": np.ascontiguousarray(xg_t[:, s]),
            "x0": np.ascontiguousarray(x0h[:, :, s]),
            "gw1": shared["gw1"], "gw2": shared["gw2"], "gw3": shared["gw3"],
            "gb12": shared["gb12"], "gb3": shared["gb3"],
            "ident": shared["ident"],
            "w0": shared["w0h"], "w1": shared["w1h"], "w2": shared["w2h"],
            "bh": shared["bh"],
        })
    return in_maps


def kernel(**inputs):
    if "nc" not in _CACHE:
        _CACHE["nc"] = build()
    nc = _CACHE["nc"]
    in_maps = _prep(inputs)
    res = run_bass_kernel_spmd(nc, in_maps, core_ids=list(range(NCORES)))
    preds, coeffs = [], []
    for ci in range(NCORES):
        r = res.results[ci]
        preds.append(np.ascontiguousarray(r["predt"].T))          # [NS, OUT]
        c4 = r["coeff4"]                                          # [128, 4, E]
        coeffs.append(np.ascontiguousarray(
            c4.transpose(1, 0, 2).reshape(NS, E)))
    pred = np.concatenate(preds, axis=0).astype(np.float32)
    coeff = np.concatenate(coeffs, axis=0).astype(np.float32)
    return pred, coeff
